# revision 48
# baseline (speedup 1.0000x reference)
"""Multi-head causal attention (B=2, S=2048, D=2048, H=16, HD=128) on 8 TRN2
NeuronCores.

Sharding: data-parallel over batch (2 groups of 4 cores) x tensor-parallel
over heads (4 heads per core).  Each core computes q/k/v projections for its
512 columns (4 heads), causal attention for those heads, and a partial
(contraction-sharded) wo product.  The 4 partial outputs per batch are summed
on the host (the "all-reduce after wo" of the sharding hint).

Everything on-chip is computed in transposed orientation:
  xT [d, s] (host pre-transposed), qT/kT [j, s], scores^T [t, s], out^T [j2, s]
so every matmul contraction lands on the partition axis with zero on-chip
transposes.

Performance structure:
  * all matmul operands are bf16 (inputs converted on host); PSUM stays fp32.
    rel err budget is 2e-2, bf16 lands ~5.5e-3.
  * all four weight matrices are SBUF-resident (one prologue DMA interleaved
    with chunk-0 x tiles so the first matmul starts ~1us in).
  * fine-grained emission interleave: within each steady-state iteration the
    projection matmuls of chunk c ("feeder" units, no cross-engine waits) are
    woven between the attention tiles of chunk c-1, so the in-order PE queue
    always has independent work while the activation engine (exp, ~926ns/tile,
    the attention critical path) catches up.  The chunk-3 attention tail is
    fed with the wo(2) matmuls the same way.
  * output is written as bf16; host upcasts and sums the partials.

Softmax uses exp without max-subtraction (scores are O(4), exact in fp32)
with causal masking by a precomputed staircase 0/1 mask applied post-exp
(exact zeros, matching the reference's exp(-1e9) == 0 underflow).  Fully
masked key blocks are skipped; denominators come from a ones-vector matmul
accumulated per key tile in PSUM.
"""

import ml_dtypes
import numpy as np

import concourse.bass as bass
import concourse.tile as tile
from concourse import bacc, mybir
from concourse.bass_utils import run_bass_kernel_spmd

B, S, D = 2, 2048, 2048
H, HD = 16, 128
P = 128
JL = 512          # local q/k/v columns per core (4 heads)
NH = 4            # heads per core
CHUNK = 512       # s-chunk
NCH = S // CHUNK  # 4
DT = D // P       # 16 d-tiles
NT = S // P       # 16 t-tiles
SCALE = 1.0 / float(np.sqrt(HD))

F32 = mybir.dt.float32
BF16 = mybir.dt.bfloat16


def build_kernel():
    nc = bacc.Bacc("TRN2", target_bir_lowering=False, debug=False, num_devices=8)
    xT = nc.dram_tensor("xT", [D, S], BF16, kind="ExternalInput").ap()
    wqT = nc.dram_tensor("wqT", [D, JL], BF16, kind="ExternalInput").ap()
    wkT = nc.dram_tensor("wkT", [D, JL], BF16, kind="ExternalInput").ap()
    wvT = nc.dram_tensor("wvT", [D, JL], BF16, kind="ExternalInput").ap()
    woT = nc.dram_tensor("woT", [JL, D], BF16, kind="ExternalInput").ap()
    outT = nc.dram_tensor("outT", [D, S], BF16, kind="ExternalOutput").ap()

    with tile.TileContext(nc) as tc:
        with (
            tc.tile_pool(name="persist", bufs=1) as persist,
            tc.tile_pool(name="xt", bufs=2) as xt_pool,
            tc.tile_pool(name="qt", bufs=3) as qt_pool,
            tc.tile_pool(name="exp", bufs=14) as exp_pool,
            tc.tile_pool(name="gs", bufs=8) as gs_pool,
            tc.tile_pool(name="ot", bufs=12) as ot_pool,
            tc.tile_pool(name="small", bufs=3) as small_pool,
            tc.tile_pool(name="osb", bufs=8) as osb_pool,
            tc.tile_pool(name="ps_main", bufs=1, space="PSUM") as ps_main,
            tc.tile_pool(name="ps_s", bufs=3, space="PSUM") as ps_s,
            tc.tile_pool(name="ps_rs", bufs=1, space="PSUM") as ps_rs,
        ):
            master_f = persist.tile([P, 896], F32, name="master_f")
            nc.gpsimd.memset(master_f[:], 1.0)
            # master[p, u] = 1.0 iff u - p - 384 >= 0 else 0.0
            nc.gpsimd.affine_select(
                out=master_f[:], in_=master_f[:], pattern=[[1, 896]],
                compare_op=mybir.AluOpType.is_ge, fill=0.0,
                base=-384, channel_multiplier=-1,
            )
            master = persist.tile([P, 896], BF16, name="master")
            nc.vector.tensor_copy(master[:], master_f[:])
            ones_f = persist.tile([P, 1], F32, name="ones_f")
            nc.vector.memset(ones_f[:], 1.0)
            ones = persist.tile([P, 1], BF16, name="ones")
            nc.vector.tensor_copy(ones[:], ones_f[:])

            # resident weights: wk tiles interleave with chunk-0 x tiles on the
            # sync queue (kproj(0) consumes them in d order); wq/wv/woT stream
            # on the scalar queue in parallel (consumed a phase later).
            wk_t = [persist.tile([P, JL], BF16, name=f"wk{d}") for d in range(DT)]
            wq_t = [persist.tile([P, JL], BF16, name=f"wq{d}") for d in range(DT)]
            wv_t = [persist.tile([P, JL], BF16, name=f"wv{d}") for d in range(DT)]
            woT_t = [persist.tile([P, D], BF16, name=f"woT{h}") for h in range(NH)]
            # both queues carry the kproj-critical tiles first (alternating d),
            # then wq, then wv/woT — strict consumption order, full bandwidth
            # on what gates the first phase.
            xt0 = []
            for d in range(DT):
                eng = nc.sync if d % 2 == 0 else nc.scalar
                eng.dma_start(out=wk_t[d][:], in_=wkT[d * P:(d + 1) * P, :])
                t_ = xt_pool.tile([P, CHUNK], BF16, name=f"xt{d}", tag=f"xt{d}")
                # halves: finer arrival granularity for the DMA-gated start
                eng.dma_start(out=t_[:, 0:CHUNK // 2],
                              in_=xT[d * P:(d + 1) * P, 0:CHUNK // 2])
                eng.dma_start(out=t_[:, CHUNK // 2:CHUNK],
                              in_=xT[d * P:(d + 1) * P, CHUNK // 2:CHUNK])
                xt0.append(t_)
            for d in range(DT):
                eng = nc.sync if d % 2 == 0 else nc.scalar
                eng.dma_start(out=wq_t[d][:], in_=wqT[d * P:(d + 1) * P, :])
            for d in range(DT):
                eng = nc.sync if d % 2 == 0 else nc.scalar
                eng.dma_start(out=wv_t[d][:], in_=wvT[d * P:(d + 1) * P, :])
            for h in range(NH):
                nc.scalar.dma_start(out=woT_t[h][:], in_=woT[h * P:(h + 1) * P, :])

            kT_t = [persist.tile([P, S], BF16, name=f"kT{h}") for h in range(NH)]
            v_t = [persist.tile([P, JL], BF16, name=f"v{t}") for t in range(NT)]

            # per-chunk state threaded through the pipeline
            xt_of = {}        # chunk -> xt tiles
            qt_of = {}        # chunk -> qt tiles
            ots_of = {}       # chunk -> normalized per-head attention outputs

            def proj_units(c):
                """Projection of chunk c as a list of emission thunks.

                Each unit is ~4 matmuls (~850ns of PE) with no cross-engine
                dependencies, so they can fill PE stalls inside attention."""
                ssl = slice(c * CHUNK, (c + 1) * CHUNK)
                units = []
                state = {}

                def k_start():
                    state["ps_k"] = [
                        ps_main.tile([P, CHUNK], F32, name=f"psk{j}", tag=f"pm{j}")
                        for j in range(4)]
                    if c == 0:
                        # chunk-0 k chains run start=False in column halves
                        # (start zeroes the whole bank), so pre-zero instead
                        for j in range(4):
                            nc.vector.memset(state["ps_k"][j][:], 0.0)
                    xt = []
                    if c == 0:
                        xt = xt0
                    else:
                        for d in range(DT):
                            t_ = xt_pool.tile([P, CHUNK], BF16, name=f"xt{d}",
                                              tag=f"xt{d}")
                            nc.sync.dma_start(
                                out=t_[:], in_=xT[d * P:(d + 1) * P, ssl])
                            xt.append(t_)
                    xt_of[c] = xt

                def k_d(d):
                    xt = xt_of[c]
                    if c == 0:
                        # chunk 0 is DMA-gated: consume x in column halves so
                        # the PE starts as soon as each half lands
                        for hs in (slice(0, CHUNK // 2),
                                   slice(CHUNK // 2, CHUNK)):
                            for j in range(4):
                                nc.tensor.matmul(
                                    state["ps_k"][j][:, hs],
                                    wk_t[d][:, j * P:(j + 1) * P],
                                    xt[d][:, hs],
                                    start=False, stop=(d == DT - 1),
                                    skip_group_check=True,
                                )
                        return
                    for j in range(4):
                        nc.tensor.matmul(
                            state["ps_k"][j][:], wk_t[d][:, j * P:(j + 1) * P],
                            xt[d][:], start=(d == 0), stop=(d == DT - 1),
                            skip_group_check=True,
                        )

                def k_copy():
                    for j in range(4):
                        nc.vector.tensor_copy(kT_t[j][:, ssl],
                                              state["ps_k"][j][:])

                def q_start():
                    state["ps_q"] = [
                        ps_main.tile([P, CHUNK], F32, name=f"psq{j}", tag=f"pm{j}")
                        for j in range(4)]

                def q_d(d):
                    xt = xt_of[c]
                    for j in range(4):
                        nc.tensor.matmul(
                            state["ps_q"][j][:], wq_t[d][:, j * P:(j + 1) * P],
                            xt[d][:], start=(d == 0), stop=(d == DT - 1),
                            skip_group_check=True,
                        )

                def q_copy():
                    qt = []
                    for j in range(4):
                        t_ = qt_pool.tile([P, CHUNK], BF16, name=f"qt{j}",
                                          tag=f"qt{j}")
                        nc.vector.tensor_copy(t_[:], state["ps_q"][j][:])
                        qt.append(t_)
                    qt_of[c] = qt

                def v_start():
                    state["ps_v"] = [
                        ps_main.tile([P, CHUNK], F32, name=f"psv{i}", tag=f"pm{i}")
                        for i in range(4)]

                def v_d(d):
                    xt = xt_of[c]
                    for i in range(4):
                        nc.tensor.matmul(
                            state["ps_v"][i][:], xt[d][:, i * P:(i + 1) * P],
                            wv_t[d][:], start=(d == 0), stop=(d == DT - 1),
                            skip_group_check=True,
                        )

                def v_copy():
                    for i in range(4):
                        nc.vector.tensor_copy(v_t[4 * c + i][:],
                                              state["ps_v"][i][:])

                units.append(k_start)
                for d in range(DT):
                    units.append(lambda d=d: k_d(d))
                units.append(k_copy)
                units.append(q_start)
                for d in range(DT):
                    units.append(lambda d=d: q_d(d))
                units.append(q_copy)
                units.append(v_start)
                for d in range(DT):
                    units.append(lambda d=d: v_d(d))
                units.append(v_copy)
                return units

            def attn_units(c, h, early_rs=False):
                """Attention for (chunk c, head h) as a list of emission
                thunks: one unit per key tile (score+exp+mask, with the
                rs/pv pair lagging 4 tiles), plus flush+finalize units."""
                s0 = c * CHUNK
                T = 4 * c + 4
                state = {}
                units = []

                G = T // 4

                def a_start():
                    state["rs"] = ps_rs.tile([1, CHUNK], F32, name="rsacc",
                                             tag="rs")
                    state["oa"] = ps_s.tile([P, CHUNK], F32, name="oacc",
                                            tag="ss")
                    state["exps"] = [None] * T
                    state["gs"] = [None] * G
                    state["rs_done"] = set()

                def emit_b(t):
                    nc.tensor.matmul(
                        state["oa"][:], v_t[t][:, h * P:(h + 1) * P],
                        state["exps"][t][:],
                        start=(t == 0), stop=(t == T - 1),
                        skip_group_check=True,
                    )

                def emit_rs(g):
                    nc.tensor.matmul(
                        state["rs"][:], ones[:], state["gs"][g][:],
                        start=(g == 0), stop=(g == G - 1),
                        skip_group_check=True,
                    )

                def a_group(g):
                    # 4 scores+exps, then the pv/rs pairs of the previous
                    # group: runs of identical instruction shapes keep the PE
                    # decode in batch mode.
                    qt = qt_of[c]
                    for i in range(4):
                        t = 4 * g + i
                        ps = ps_s.tile([P, CHUNK], F32, name="pss", tag="ss")
                        nc.tensor.matmul(
                            ps[:], kT_t[h][:, t * P:(t + 1) * P], qt[h][:],
                            start=True, stop=True, skip_group_check=True,
                        )
                        e = exp_pool.tile([P, CHUNK], BF16, name="exp",
                                          tag="exp")
                        nc.scalar.activation(
                            e[:], ps[:], mybir.ActivationFunctionType.Exp,
                            scale=SCALE,
                        )
                        state["exps"][t] = e
                    # per-group denominator partial sum on the vector engine
                    # (cuts the PE ones-matmul count 4x); group-local, no
                    # cross-group serial chain.
                    gs = gs_pool.tile([P, CHUNK], BF16, name="gs", tag="gs")
                    ex = state["exps"]
                    nc.vector.tensor_add(gs[:], ex[4 * g][:], ex[4 * g + 1][:])
                    nc.vector.tensor_add(gs[:], gs[:], ex[4 * g + 2][:])
                    nc.vector.tensor_add(gs[:], gs[:], ex[4 * g + 3][:])
                    state["gs"][g] = gs
                    if g >= 1:
                        for i in range(4):
                            emit_b(4 * (g - 1) + i)
                    if early_rs and g >= 2:
                        emit_rs(g - 2)
                        state["rs_done"].add(g - 2)

                def a_group_diag(g):
                    # last (diagonal) group: queries in the chunk's first half
                    # see only k-tiles 4c/4c+1, so tiles 4c+2/4c+3 run at
                    # half width on q[256:512], packed side by side into one
                    # PSUM bank (start=True zeroes the whole bank, so the
                    # second half accumulates onto zeros) -> one shared exp.
                    qt = qt_of[c]
                    HC = CHUNK // 2
                    t0 = 4 * g
                    for i in range(2):
                        t = t0 + i
                        ps = ps_s.tile([P, CHUNK], F32, name="pss", tag="ss")
                        nc.tensor.matmul(
                            ps[:], kT_t[h][:, t * P:(t + 1) * P], qt[h][:],
                            start=True, stop=True, skip_group_check=True,
                        )
                        e = exp_pool.tile([P, CHUNK], BF16, name="exp",
                                          tag="exp")
                        nc.scalar.activation(
                            e[:], ps[:], mybir.ActivationFunctionType.Exp,
                            scale=SCALE,
                        )
                        off = 384 + s0 - t * P
                        nc.vector.tensor_mul(
                            e[:], e[:], master[:, off:off + CHUNK])
                        state["exps"][t] = e
                    psp = ps_s.tile([P, CHUNK], F32, name="pss", tag="ss")
                    nc.tensor.matmul(
                        psp[:, 0:HC],
                        kT_t[h][:, (t0 + 2) * P:(t0 + 3) * P],
                        qt[h][:, HC:CHUNK],
                        start=True, stop=True, skip_group_check=True,
                    )
                    nc.tensor.matmul(
                        psp[:, HC:CHUNK],
                        kT_t[h][:, (t0 + 3) * P:(t0 + 4) * P],
                        qt[h][:, HC:CHUNK],
                        start=False, stop=True, skip_group_check=True,
                    )
                    ep = exp_pool.tile([P, CHUNK], BF16, name="exp", tag="exp")
                    nc.scalar.activation(
                        ep[:], psp[:], mybir.ActivationFunctionType.Exp,
                        scale=SCALE,
                    )
                    # mask offsets: off = 384 + s0 + qa - t*128 with qa=256
                    off2 = 384 + HC - 2 * P
                    off3 = 384 + HC - 3 * P
                    nc.vector.tensor_mul(ep[:, 0:HC], ep[:, 0:HC],
                                         master[:, off2:off2 + HC])
                    nc.vector.tensor_mul(ep[:, HC:CHUNK], ep[:, HC:CHUNK],
                                         master[:, off3:off3 + HC])
                    state["ep"] = ep
                    # denominator partials: full tiles add at full width, the
                    # packed halves both belong to q[256:512]
                    gs = gs_pool.tile([P, CHUNK], BF16, name="gs", tag="gs")
                    ex = state["exps"]
                    nc.vector.tensor_add(gs[:], ex[t0][:], ex[t0 + 1][:])
                    nc.vector.tensor_add(gs[:, HC:CHUNK], gs[:, HC:CHUNK],
                                         ep[:, 0:HC])
                    nc.vector.tensor_add(gs[:, HC:CHUNK], gs[:, HC:CHUNK],
                                         ep[:, HC:CHUNK])
                    state["gs"][g] = gs
                    if g >= 1:
                        for i in range(4):
                            emit_b(4 * (g - 1) + i)
                    if early_rs and g >= 2:
                        emit_rs(g - 2)
                        state["rs_done"].add(g - 2)

                def a_flush(g):
                    # pv of the diagonal group: two full tiles, then the two
                    # packed halves into the o_acc columns of q[256:512]
                    HC = CHUNK // 2
                    t0 = 4 * g
                    ex = state["exps"]
                    ep = state["ep"]
                    oa = state["oa"]
                    nc.tensor.matmul(
                        oa[:], v_t[t0][:, h * P:(h + 1) * P], ex[t0][:],
                        start=(t0 == 0), stop=False, skip_group_check=True,
                    )
                    nc.tensor.matmul(
                        oa[:], v_t[t0 + 1][:, h * P:(h + 1) * P],
                        ex[t0 + 1][:],
                        start=False, stop=False, skip_group_check=True,
                    )
                    nc.tensor.matmul(
                        oa[:, HC:CHUNK], v_t[t0 + 2][:, h * P:(h + 1) * P],
                        ep[:, 0:HC],
                        start=False, stop=False, skip_group_check=True,
                    )
                    nc.tensor.matmul(
                        oa[:, HC:CHUNK], v_t[t0 + 3][:, h * P:(h + 1) * P],
                        ep[:, HC:CHUNK],
                        start=False, stop=True, skip_group_check=True,
                    )
                    # rs matmuls for the whole head: the group sums are long
                    # done by now, so the PE never waits on them.
                    for gg in range(G):
                        if gg not in state["rs_done"]:
                            emit_rs(gg)

                def a_fin():
                    rs_sb = small_pool.tile([1, CHUNK], F32, name="rssb",
                                            tag="rssb")
                    nc.vector.reciprocal_approx_fast(out=rs_sb[:],
                                                     in_=state["rs"][:])
                    rb = small_pool.tile([P, CHUNK], F32, name="rb", tag="rb")
                    nc.gpsimd.partition_broadcast(rb[:], rs_sb[:])
                    ot = ot_pool.tile([P, CHUNK], BF16, name="ot", tag="ot")
                    nc.vector.tensor_mul(ot[:], state["oa"][:], rb[:])
                    ots_of.setdefault(c, []).append(ot)

                units.append(a_start)
                for g in range(G - 1):
                    units.append(lambda g=g: a_group(g))
                units.append(lambda: a_group_diag(G - 1))
                units.append(lambda: a_flush(G - 1))
                units.append(a_fin)
                return units

            def wo_units(c, use_pm=False):
                ssl = slice(c * CHUNK, (c + 1) * CHUNK)
                units = []

                def w_j2(j2):
                    ots = ots_of[c]
                    if use_pm:
                        pw = ps_main.tile([P, CHUNK], F32, name="pw",
                                          tag=f"pm{j2 % 4}")
                    else:
                        pw = ps_s.tile([P, CHUNK], F32, name="pw", tag="ss")
                    for h in range(NH):
                        nc.tensor.matmul(
                            pw[:], woT_t[h][:, j2 * P:(j2 + 1) * P], ots[h][:],
                            start=(h == 0), stop=(h == NH - 1),
                            skip_group_check=True,
                        )
                    ob = osb_pool.tile([P, CHUNK], BF16, name="ob", tag="ob")
                    nc.vector.tensor_copy(ob[:], pw[:])
                    # mid-kernel stores stay off the sync queue (it carries the
                    # next chunk's x loads); the tail alternates queues so the
                    # final output drain uses both DMA paths
                    if c < NCH - 2:
                        eng = nc.scalar
                    else:
                        eng = nc.scalar if j2 % 2 == 0 else nc.sync
                    eng.dma_start(out=outT[j2 * P:(j2 + 1) * P, ssl],
                                  in_=ob[:])

                for j2 in range(DT):
                    units.append(lambda j2=j2: w_j2(j2))
                return units

            def run(units):
                for u in units:
                    u()

            # ---- software pipeline: proj(c) interleaved with attn(c-1) at
            # phase granularity (k / q / v thirds between pairs of heads) ----
            pu0 = proj_units(0)
            run(pu0)
            for c in range(1, NCH):
                pu = proj_units(c)
                third = (len(pu) + 2) // 3
                run(pu[:third])                       # kproj(c)
                run(attn_units(c - 1, 0))
                run(attn_units(c - 1, 1))
                run(pu[third:2 * third])              # qproj(c)
                run(attn_units(c - 1, 2))
                run(attn_units(c - 1, 3))
                run(pu[2 * third:])                   # vproj(c)
                if c - 1 < NCH - 2:
                    run(wo_units(c - 1))
            # tail: wo(2) blocks woven between the chunk-3 attention groups
            # (they land in the pm banks, idle once projections are done)
            wo2 = wo_units(NCH - 2, use_pm=True)
            tail = []
            for h in range(NH):
                tail.extend(attn_units(NCH - 1, h, early_rs=(h == NH - 1)))
            wi = 0
            for i, u in enumerate(tail):
                u()
                take = (i * len(wo2)) // len(tail) + 1
                while wi < len(wo2) and wi < take:
                    wo2[wi]()
                    wi += 1
            while wi < len(wo2):
                wo2[wi]()
                wi += 1
            run(wo_units(NCH - 1, use_pm=True))

    nc.compile()
    return nc


_NC_CACHE = None


def _get_nc():
    global _NC_CACHE
    if _NC_CACHE is None:
        _NC_CACHE = build_kernel()
    return _NC_CACHE


def make_in_maps(x, wq, wk, wv, wo):
    bf = ml_dtypes.bfloat16
    in_maps = []
    for core in range(8):
        b, g = core // 4, core % 4
        j0 = g * JL
        in_maps.append({
            "xT": np.ascontiguousarray(x[b].T).astype(bf),
            "wqT": np.ascontiguousarray(wq[j0:j0 + JL, :].T).astype(bf),
            "wkT": np.ascontiguousarray(wk[j0:j0 + JL, :].T).astype(bf),
            "wvT": np.ascontiguousarray(wv[j0:j0 + JL, :].T).astype(bf),
            "woT": np.ascontiguousarray(wo[:, j0:j0 + JL].T).astype(bf),
        })
    return in_maps


def kernel(x, freqs_complex=None, mask=None, wq=None, wk=None, wv=None, wo=None,
           **_unused):
    x = np.asarray(x, dtype=np.float32)
    wq = np.asarray(wq, dtype=np.float32)
    wk = np.asarray(wk, dtype=np.float32)
    wv = np.asarray(wv, dtype=np.float32)
    wo = np.asarray(wo, dtype=np.float32)

    nc = _get_nc()
    in_maps = make_in_maps(x, wq, wk, wv, wo)
    res = run_bass_kernel_spmd(nc, in_maps, list(range(8)))

    out = np.zeros((B, S, D), dtype=np.float32)
    for core in range(8):
        out[core // 4] += res.results[core]["outT"].T.astype(np.float32)
    return out
